# revision 19
# baseline (speedup 1.0000x reference)
"""MoE layer (top-2 of 8 experts, D=1024, H=4096) on 8 Trainium2 NeuronCores.

Strategy (expert-parallel, per sharding hint):
  - Routing (softmax top-2 over 8 experts) computed on host in float64 from
    the full inputs; tokens are gathered per expert ("all-to-all by routing
    decision" done as part of input sharding).
  - Core e runs expert e's MLP on its routed tokens:
        y = gelu_tanh(x @ W1[e] + b1[e]) @ W2[e]
    as a single Bass/Tile kernel: matmuls in float32r (full PE rate, ~1e-4
    rel err), gelu on the ACT LUT, fp32 PSUM accumulation.
  - Host combines: out[t] = sum_e w[t,e] * (y_e[t] + b2[e]).

Device kernel layout (per core):
  xT [D, CAP] resident in SBUF; W2 [H, D] resident; W1 streamed per
  (chunk, mh). Tokens processed in chunks of 384 so the three mm2 PSUM
  accumulators ([128,1024] = 2 banks each) plus the mm1 accumulator
  ([128,384] = 1 bank, double-buffered) exactly fill the 8 PSUM banks.
  mm1 produces hT [H-tile, tokens] directly (no transposes anywhere), and
  mm2 consumes each hT tile right after its gelu, so hT never needs to be
  fully resident.
"""

import os
import numpy as np

P = 128
CHUNK = 384  # tokens per chunk; 3 mm2 psum tiles * 2 banks + 2 mm1 banks = 8

_BUILD_CACHE = {}
LAST_RESULTS = None  # BassKernelResults of the most recent run (for test.py)


def _routing(x2d, Wg):
    """float64 softmax top-2 routing. Returns (weights [T,E], top2 [T,2])."""
    logits = x2d.astype(np.float64) @ Wg.astype(np.float64)
    m = logits.max(axis=1, keepdims=True)
    p = np.exp(logits - m)
    p /= p.sum(axis=1, keepdims=True)
    top2 = np.argpartition(-p, 2, axis=1)[:, :2]
    w = np.zeros_like(p)
    np.put_along_axis(w, top2, np.take_along_axis(p, top2, axis=1), axis=1)
    return w, top2


GROUP = 4  # H-tiles per group; W1/W2 each stream exactly once


def _build(D, H, cap, repeat=1):
    import concourse.bacc as bacc
    import concourse.mybir as mybir
    import concourse.tile as tile

    key = (D, H, cap, repeat)
    if key in _BUILD_CACHE:
        return _BUILD_CACHE[key]

    kd = D // P            # mm1 contraction tiles (over D)
    mh_tiles = H // P      # H tiles (mm1 output partitions / mm2 contraction)
    ngroups = mh_tiles // GROUP
    nd = D // 512          # mm2 output free-dim slices
    mt_tiles = cap // P    # token tiles
    # token n-slices for mm1 (<=512 each, f32r needs >=256 for full rate)
    ntiles = []
    off = 0
    while off < cap:
        w = min(512, cap - off)
        ntiles.append((off, w))
        off += w

    nc = bacc.Bacc("TRN2", target_bir_lowering=False, debug=False, num_devices=8)
    f32 = mybir.dt.float32
    f32r = mybir.dt.float32r

    d_xT = nc.dram_tensor("xT", [D, cap], f32r, kind="ExternalInput")
    # host-packed W1: w1p[mh, p, ko*128+j] = W1[ko*128+p, mh*128+j]
    d_w1 = nc.dram_tensor("w1p", [mh_tiles, P, D], f32r, kind="ExternalInput")
    d_w2 = nc.dram_tensor("w2", [H, D], f32r, kind="ExternalInput")
    d_b1 = nc.dram_tensor("b1", [H], f32, kind="ExternalInput")
    d_y = nc.dram_tensor("y", [cap, D], f32, kind="ExternalOutput")

    xT_t = d_xT.rearrange("(ko p) n -> p ko n", p=P)
    w2_t = d_w2.rearrange("(ko p) d -> p ko d", p=P)
    b1_t = d_b1.rearrange("(ko p) -> p ko", p=P)

    gelu = mybir.ActivationFunctionType.Gelu_apprx_tanh
    add_op = mybir.AluOpType.add

    with tile.TileContext(nc) as tc:
        with (
            tc.tile_pool(name="resident", bufs=1) as res,
            tc.tile_pool(name="w1s", bufs=2 * GROUP) as w1pool,
            tc.tile_pool(name="w2s", bufs=2 * GROUP) as w2pool,
            tc.tile_pool(name="ht", bufs=2) as htpool,
            tc.tile_pool(name="ph", bufs=3, space="PSUM") as phpool,
            tc.tile_pool(name="py", bufs=3, space="PSUM") as pypool,
        ):
          for rep in range(repeat):
            # resident: xT, b1, y accumulator
            xT_sb = res.tile([P, kd, cap], f32r, tag="xT", name=f"xT_{rep}")
            for k in range(kd):
                nc.sync.dma_start(xT_sb[:, k, :], xT_t[:, k, :])
            b1_sb = res.tile([P, mh_tiles], f32, tag="b1", name=f"b1_{rep}")
            nc.sync.dma_start(b1_sb[:], b1_t[:, :])
            yacc = res.tile([P, mt_tiles, D], f32, tag="yacc", name=f"yacc_{rep}")

            for g in range(ngroups):
                # stream this group's weights (each byte of W1/W2 read once)
                w1g, w2g = [], []
                for mi in range(GROUP):
                    mh = g * GROUP + mi
                    w1t = w1pool.tile([P, kd, P], f32r, tag="w1",
                                      name=f"w1_{rep}_{mh}")
                    nc.sync.dma_start(
                        w1t[:], d_w1[mh].rearrange("p (ko j) -> p ko j", ko=kd))
                    w1g.append(w1t)
                    w2t = w2pool.tile([P, D], f32r, tag="w2",
                                      name=f"w2_{rep}_{mh}")
                    nc.sync.dma_start(w2t[:], w2_t[:, mh, :])
                    w2g.append(w2t)

                # mm1 + gelu: hT for the group's 8 H-tiles, all tokens
                htg = htpool.tile([P, GROUP, cap], f32r, tag="ht",
                                  name=f"ht_{rep}_{g}")
                for mi in range(GROUP):
                    mh = g * GROUP + mi
                    for (n0, nw) in ntiles:
                        ph = phpool.tile([P, 512], f32, tag="ph",
                                         name=f"ph_{rep}_{mh}_{n0}")
                        for k in range(kd):
                            nc.tensor.matmul(
                                ph[:, :nw], w1g[mi][:, k, :],
                                xT_sb[:, k, n0:n0 + nw],
                                start=(k == 0), stop=(k == kd - 1),
                            )
                        nc.scalar.activation(
                            htg[:, mi, n0:n0 + nw], ph[:, :nw], gelu,
                            bias=b1_sb[:, mh:mh + 1],
                        )

                # mm2: accumulate this group's contribution into yacc
                for mt in range(mt_tiles):
                    for n in range(nd):
                        py = pypool.tile([P, 512], f32, tag="py",
                                         name=f"py_{rep}_{g}_{mt}_{n}")
                        for mi in range(GROUP):
                            nc.tensor.matmul(
                                py[:], htg[:, mi, mt * P:(mt + 1) * P],
                                w2g[mi][:, n * 512:(n + 1) * 512],
                                start=(mi == 0), stop=(mi == GROUP - 1),
                            )
                        dst = yacc[:, mt, n * 512:(n + 1) * 512]
                        if g == 0:
                            nc.vector.tensor_copy(dst, py[:])
                        else:
                            nc.vector.tensor_tensor(dst, dst, py[:], add_op)

            for mt in range(mt_tiles):
                nc.sync.dma_start(d_y[mt * P:(mt + 1) * P, :], yacc[:, mt, :])

    nc.finalize()
    _BUILD_CACHE[key] = nc
    return nc


def _pjrt_plumbing(nc):
    """Names/avals/zero-outs for driving nc through _bass_exec_p ourselves.

    We bypass run_bass_kernel_spmd so the inputs can be device_put into
    device HBM first — host-backed buffers are re-streamed over the slow
    host link on every NEFF execution otherwise.
    """
    import jax
    import concourse.bass2jax as b2j
    import concourse.mybir as mybir

    b2j.install_neuronx_cc_hook()
    partition_name = nc.partition_id_tensor.name if nc.partition_id_tensor else None
    in_names, out_names, out_avals, zero_outs = [], [], [], []
    for alloc in nc.m.functions[0].allocations:
        if not isinstance(alloc, mybir.MemoryLocationSet):
            continue
        name = alloc.memorylocations[0].name
        if alloc.kind == "ExternalInput":
            if name != partition_name:
                in_names.append(name)
        elif alloc.kind == "ExternalOutput":
            out_names.append(name)
            shape = tuple(alloc.tensor_shape)
            dtype = mybir.dt.np(alloc.dtype)
            out_avals.append(jax.core.ShapedArray(shape, dtype))
            zero_outs.append(np.zeros(shape, dtype))
    return partition_name, in_names, out_names, out_avals, zero_outs


def _make_fn(nc, n_iters=1):
    """Jitted 8-core shard_map callable running the NEFF n_iters times."""
    import jax
    from jax.sharding import Mesh, PartitionSpec
    from jax.experimental.shard_map import shard_map
    import concourse.bass2jax as b2j

    partition_name, in_names, out_names, out_avals, zero_outs = _pjrt_plumbing(nc)
    all_names = in_names + out_names
    if partition_name is not None:
        all_names = all_names + [partition_name]

    def _body(*args):
        operands = list(args)
        if partition_name is not None:
            operands.append(b2j.partition_id_tensor())
        outs = None
        for _ in range(n_iters):
            outs = b2j._bass_exec_p.bind(
                *operands,
                out_avals=tuple(out_avals),
                in_names=tuple(all_names),
                out_names=tuple(out_names),
                lowering_input_output_aliases=(),
                sim_require_finite=True,
                sim_require_nnan=True,
                nc=nc,
            )
        return tuple(outs)

    devices = jax.devices()[:8]
    mesh = Mesh(np.asarray(devices), ("core",))
    nin = len(in_names) + len(out_names)
    fn = jax.jit(shard_map(
        _body, mesh=mesh,
        in_specs=(PartitionSpec("core"),) * nin,
        out_specs=(PartitionSpec("core"),) * len(out_names),
        check_rep=False,
    ))
    return fn, mesh, in_names, out_names, zero_outs


def _run_spmd(nc, in_maps):
    """Run the finalized nc once on 8 cores with device-resident inputs.

    Returns (list of per-core {out_name: np.ndarray}, device_inputs) so a
    caller can re-run/bench with the same device buffers.
    """
    import jax
    from jax.sharding import NamedSharding, PartitionSpec

    fn, mesh, in_names, out_names, zero_outs = _make_fn(nc, 1)
    concat_in = [
        np.concatenate([np.asarray(in_maps[c][nm]) for c in range(8)], axis=0)
        for nm in in_names
    ] + [np.concatenate([z] * 8, axis=0) for z in zero_outs]
    sharding = NamedSharding(mesh, PartitionSpec("core"))
    dev_in = [jax.device_put(a, sharding) for a in concat_in]
    jax.block_until_ready(dev_in)
    outs = fn(*dev_in)
    jax.block_until_ready(outs)
    results = []
    for c in range(8):
        r = {}
        for i, nm in enumerate(out_names):
            full = np.asarray(outs[i])
            per = full.shape[0] // 8
            r[nm] = full[c * per:(c + 1) * per]
        results.append(r)
    return results, dev_in


def bench_exec_ns(D, H, cap, in_maps, k2=33, nsets=4, reps=16):
    """Per-execution NEFF time (ns): repeat the kernel body K times inside one
    NEFF and take the median wall-time difference. Distinct input sets cycle
    between calls to defeat identical-call caching in the dispatch path."""
    import time as _time
    import jax
    from jax.sharding import NamedSharding, PartitionSpec

    def stage(nc):
        fn, mesh, in_names, out_names, zero_outs = _make_fn(nc, 1)
        sharding = NamedSharding(mesh, PartitionSpec("core"))
        sets = []
        for s in range(nsets):
            scale = 1.0 + 0.125 * s
            concat = [
                np.concatenate(
                    [np.asarray(in_maps[c][nm]) for c in range(8)], axis=0
                ) * scale
                for nm in in_names
            ] + [np.concatenate([z] * 8, axis=0) for z in zero_outs]
            dev = [jax.device_put(a, sharding) for a in concat]
            jax.block_until_ready(dev)
            sets.append(dev)
        return fn, sets

    def measure(repeat):
        nc = _build(D, H, cap, repeat=repeat)
        fn, sets = stage(nc)
        jax.block_until_ready(fn(*sets[0]))
        walls = []
        for i in range(reps):
            dev = sets[i % nsets]
            t0 = _time.perf_counter()
            jax.block_until_ready(fn(*dev))
            walls.append(_time.perf_counter() - t0)
        walls.sort()
        return walls[len(walls) // 2]

    m1 = measure(1)
    mk = measure(k2)
    return (mk - m1) / (k2 - 1) * 1e9


def kernel(x, Wg, W1, b1, W2, b2):
    global LAST_RESULTS

    x = np.asarray(x, dtype=np.float32)
    Wg = np.asarray(Wg, dtype=np.float32)
    W1 = np.asarray(W1, dtype=np.float32)
    b1 = np.asarray(b1, dtype=np.float32)
    W2 = np.asarray(W2, dtype=np.float32)
    b2 = np.asarray(b2, dtype=np.float32)

    B, S, D = x.shape
    E, _, H = W1.shape
    T = B * S
    x2d = np.ascontiguousarray(x.reshape(T, D))

    weights, _top2 = _routing(x2d, Wg)

    idx = [np.nonzero(weights[:, e])[0] for e in range(E)]
    maxn = max(len(i) for i in idx)
    cap = max(P, -(-maxn // P) * P)

    nc = _build(D, H, cap)

    kd, mh_tiles = D // P, H // P
    in_maps = []
    for e in range(E):
        xT = np.zeros((D, cap), dtype=np.float32)
        xT[:, :len(idx[e])] = x2d[idx[e]].T
        # pack W1 so each [P, kd*P] H-block is contiguous per partition:
        # w1p[mh, p, ko*P + j] = W1[e, ko*P + p, mh*P + j]
        w1p = np.ascontiguousarray(
            W1[e].reshape(kd, P, mh_tiles, P)
                 .transpose(2, 1, 0, 3)
                 .reshape(mh_tiles, P, D))
        in_maps.append({
            "xT": xT,
            "w1p": w1p,
            "w2": np.ascontiguousarray(W2[e]),
            "b1": np.ascontiguousarray(b1[e]),
        })

    results, dev_in = _run_spmd(nc, in_maps)
    LAST_RESULTS = {
        "nc": nc, "dev_in": dev_in, "results": results,
        "D": D, "H": H, "cap": cap, "in_maps": in_maps,
    }

    out = weights.astype(np.float32) @ b2  # the b2 term, exact
    out = out.astype(np.float64)
    for e in range(E):
        y_e = results[e]["y"][:len(idx[e])].astype(np.float64)
        out[idx[e]] += weights[idx[e], e][:, None] * y_e
    return out.reshape(B, S, D).astype(np.float32)
